# revision 1
# baseline (speedup 1.0000x reference)
"""Trainium2 Bass kernel v2 for nn_BattleModel (segment_reduce).

Architecture (per core; 2048 contiguous segments/core, SPMD-identical
program across 8 cores):

- Unit MLP h=relu(x@W1+b1) on TensorE as K=20 fp16 matmuls: each rhs
  column carries TWO units (A at lanes 0:10 -> out rows 0:64, B at lanes
  10:20 -> out rows 64:128).  Four 20-lane bands at partition pitch 32
  pack the rhs to ~2B/col/partition.
- Transit+first pool level fused: physical slots come in (A-bank, B-bank)
  512-col pairs.  ACT relu-evicts the B bank into a small f16 ring; DVE
  scalar_tensor_tensor computes h16[alpha] = relu(psumA) + ringB in one
  1x pass.  This is the only PSUM->SBUF transit (GPSIMD/DMA cannot read
  PSUM).
- Segment pooling: segments are rank-sorted by a shared key
  K=max(ceil(cL/2),ceil(cR/2)) with a cross-core envelope so all cores
  run one program.  In-place ragged halving trees on DVE (f16 2x) /
  GPSIMD run in alpha space; the last level folds halves into a
  [128, 2, 2048, 4] pooled buffer (4 cols/segment).
- Combine MLP: Wc1 rows are duplicated so the A+B partition fold rides
  the PSUM accumulation; 4 pooled cols/segment stream as 4 accumulating
  matmul passes per side.  relu+sigmoid on ACT, f16 out, host inverse
  permutation.
"""

from contextlib import ExitStack

import numpy as np

import concourse.bacc as bacc
import concourse.bass as bass
import concourse.tile as tile
from concourse import mybir
from concourse.bass_utils import run_bass_kernel_spmd

N_UNITS = 524288
BATCH = 16384
N_CORES = 8
SEG_PER_CORE = BATCH // N_CORES  # 2048
NFEAT = 9
FOLD = 4  # pooled columns per segment fed to the combine matmul
LATE_THR = 0.93  # groups ending after this fraction of alpha may use DVE
ALT_MOD = 3      # 1-in-ALT_MOD late tree pieces go to GPSIMD
TREE_CHUNK_G = 1280
RING_BUFS = 6
PRELOAD_SCALE = 1.0

# eviction mode per supertile index modulo len: True -> STT fusion
# (ACT evicts B, DVE fuses A), False -> plain (ACT evicts both, DVE does
# the pair add at f16 2x).
STT_PATTERN = [True] * 17 + [False]


def _side_counts(seg, core):
    lo = int(np.searchsorted(seg, SEG_PER_CORE * core, side="left"))
    hi = int(np.searchsorted(seg, SEG_PER_CORE * (core + 1), side="left"))
    counts = np.bincount(
        seg[lo:hi] - SEG_PER_CORE * core, minlength=SEG_PER_CORE
    ).astype(np.int64)
    return lo, counts


def host_prep(left_feats, right_feats, left_seg, right_seg):
    left_feats = np.asarray(left_feats, dtype=np.float32)
    right_feats = np.asarray(right_feats, dtype=np.float32)
    left_seg = np.asarray(left_seg)
    right_seg = np.asarray(right_seg)

    cores = []
    for d in range(N_CORES):
        loL, cL = _side_counts(left_seg, d)
        loR, cR = _side_counts(right_seg, d)
        K = np.maximum(np.maximum((cL + 1) // 2, (cR + 1) // 2), 1)
        order = np.argsort(-K, kind="stable")  # big segments first
        cores.append(dict(loL=loL, loR=loR, cL=cL, cR=cR, K=K, order=order,
                          K_sorted=K[order]))

    K_env = np.zeros(SEG_PER_CORE, dtype=np.int64)
    for c in cores:
        K_env = np.maximum(K_env, c["K_sorted"])
    K_env = np.maximum(K_env, 7)  # m' >= 4 so the fold-4 endgame is uniform
    m_env = (K_env + 1) // 2  # A-cols per segment
    abase = np.zeros(SEG_PER_CORE, dtype=np.int64)
    abase[1:] = np.cumsum(m_env)[:-1]
    A_tot = int(m_env.sum())
    A2 = -(-A_tot // 512) * 512  # alpha capacity, 512-multiple
    n_st = A2 // 512  # supertiles per side (each: A bank + B bank)
    phys_per_side = 2 * A2

    # 4 bands of whole supertiles
    q, rem = divmod(n_st, 4)
    st_per_band = [q + (1 if b < rem else 0) for b in range(4)]
    st_edges = np.concatenate([[0], np.cumsum(st_per_band)])  # supertile idx
    W_b = [st_per_band[b] * 1024 for b in range(4)]
    W0 = max(W_b)

    groups = []
    j = 0
    while j < SEG_PER_CORE:
        j2 = j
        while j2 < SEG_PER_CORE and m_env[j2] == m_env[j]:
            j2 += 1
        groups.append((int(m_env[j]), j, j2 - j, int(abase[j])))
        j = j2

    st_of_phys = lambda P: P // 1024

    for c in cores:
        order = c["order"]
        for side in ("L", "R"):
            feats = left_feats if side == "L" else right_feats
            lo = c["loL"] if side == "L" else c["loR"]
            cnt = c["cL"] if side == "L" else c["cR"]
            src_start = np.zeros(SEG_PER_CORE, dtype=np.int64)
            src_start[1:] = np.cumsum(cnt)[:-1]
            cnt_sorted = cnt[order]
            src_start_sorted = src_start[order]
            tot = int(cnt_sorted.sum())
            # i = index of unit within its (rank-sorted) segment
            i_in_seg = np.arange(tot) - np.repeat(
                np.cumsum(cnt_sorted) - cnt_sorted, cnt_sorted
            )
            src_row = lo + np.repeat(src_start_sorted, cnt_sorted) + i_in_seg
            pair = i_in_seg // 2
            half = i_in_seg % 2
            me = np.repeat(m_env, cnt_sorted)
            ab = np.repeat(abase, cnt_sorted)
            in_A = pair < me
            alpha = ab + np.where(in_A, pair, pair - me)
            P = 1024 * (alpha // 512) + (alpha % 512) + np.where(in_A, 0, 512)
            st = P // 1024
            band = np.searchsorted(st_edges, st, side="right") - 1
            col = P - st_edges[band] * 1024
            row0 = 20 * band + 10 * half
            xh = np.zeros((80, W0), dtype=np.float16)
            f32feats = feats[src_row]  # [tot, 9]
            for f in range(NFEAT):
                xh[row0 + f, col] = f32feats[:, f].astype(np.float16)
            xh[row0 + NFEAT, col] = 1.0
            c["xh" + side] = xh

    aux = (st_edges, W_b, W0)
    # legacy key aliases (upc/m_pad/poolw) keep older harness fallbacks
    # that call build_nc(meta["upc"], meta["m_pad"], meta["poolw"],
    # meta["groups"]) working
    return dict(A2=A2, n_st=n_st, aux=aux, groups=groups, cores=cores,
                upc=A2, m_pad=n_st, poolw=aux)


def make_weight_arrays(W1, b1, Wc1, bc1, Wc2, bc2):
    W1p = np.concatenate(
        [np.asarray(W1, np.float32), np.asarray(b1, np.float32)[None, :]], axis=0
    )  # [10, 64]
    Wc1 = np.asarray(Wc1, np.float32)
    wts = np.zeros((128, 512), dtype=np.float16)
    for b in range(4):
        for k in range(10):
            wts[32 * b + k, 0:64] = W1p[k].astype(np.float16)
            wts[32 * b + 10 + k, 64:128] = W1p[k].astype(np.float16)
    # combine lhsT blocks padded to 128 outputs (cols 32:128 zero)
    wts[0:64, 128:160] = Wc1[0:64].astype(np.float16)
    wts[64:128, 128:160] = Wc1[0:64].astype(np.float16)
    wts[0:64, 256:288] = Wc1[64:128].astype(np.float16)
    wts[64:128, 256:288] = Wc1[64:128].astype(np.float16)
    wts[0:32, 384] = np.asarray(Wc2, np.float32)[:, 0].astype(np.float16)
    bias = np.zeros((128, 2), dtype=np.float32)
    bias[0:32, 0] = np.asarray(bc1, np.float32)
    bias[0, 1] = np.asarray(bc2, np.float32)[0]
    return dict(wts=wts, bias=bias)


# ------------------------------------------------------------- bass program

def build_nc(A2, n_st, aux, groups):
    st_edges, W_b, W0 = aux
    f16, f32 = mybir.dt.float16, mybir.dt.float32
    nc = bacc.Bacc()
    relu = mybir.ActivationFunctionType.Relu
    sigmoid = mybir.ActivationFunctionType.Sigmoid
    add = mybir.AluOpType.add
    mx = mybir.AluOpType.max

    xh_dram = {
        s: nc.declare_dram_parameter("xh" + s, [80, W0], f16, isOutput=False)
        for s in ("L", "R")
    }
    wts_dram = nc.declare_dram_parameter("wts", [128, 512], f16, isOutput=False)
    bias_dram = nc.declare_dram_parameter("bias", [128, 2], f32, isOutput=False)
    out_dram = nc.declare_dram_parameter("out", [1, SEG_PER_CORE], f16,
                                         isOutput=True)

    # tree engine balance: DVE is preloaded with the transit cost so the
    # greedy pushes nearly all tree work onto the otherwise-idle GPSIMD.
    n_stt = sum(1 for i in range(2 * n_st) if STT_PATTERN[i % len(STT_PATTERN)])
    eng_ns = {"dve": PRELOAD_SCALE * (658.0 * n_stt + 336.0 * (2 * n_st - n_stt)), "gps": 0.0}

    TREE_CHUNK = TREE_CHUNK_G  # max free-size per tree op

    def _tree_tt_one(out_ap, in0_ap, in1_ap, cols, contig, dve_ok=True,
                     force_dve=False):
        dve_cost = cols * (0.52 if contig else 1.042) + 70
        gps_cost = cols * 1.98 + 130
        if force_dve == "alt":
            eng_ns["late_i"] = eng_ns.get("late_i", 0) + 1
            force_dve = (eng_ns["late_i"] % ALT_MOD) != 0
        if not force_dve and (
            not dve_ok or eng_ns["gps"] + gps_cost < eng_ns["dve"] + dve_cost
        ):
            eng_ns["gps"] += gps_cost
            nc.gpsimd.tensor_tensor(out_ap, in0_ap, in1_ap, add)
        else:
            eng_ns["dve"] += dve_cost
            nc.vector.tensor_tensor(out_ap, in0_ap, in1_ap, add)

    def tree_tt(out_ap, in0_ap, in1_ap, cols, contig, n=None, dve_ok=True,
                force_dve=False):
        # APs shaped [p, s, n, a]; split along n when the op is large
        if n is None or cols <= TREE_CHUNK:
            _tree_tt_one(out_ap, in0_ap, in1_ap, cols, contig, dve_ok,
                         force_dve)
            return
        pieces = -(-cols // TREE_CHUNK)
        step = -(-n // pieces)
        for n0 in range(0, n, step):
            n1 = min(n, n0 + step)
            _tree_tt_one(
                out_ap[:, :, n0:n1], in0_ap[:, :, n0:n1], in1_ap[:, :, n0:n1],
                cols * (n1 - n0) // n, contig, dve_ok, force_dve,
            )

    with tile.TileContext(nc) as tc, ExitStack() as ctx:
        consts = ctx.enter_context(tc.tile_pool(name="consts", bufs=1))
        big = ctx.enter_context(tc.tile_pool(name="big", bufs=1))
        ring_pool = ctx.enter_context(tc.tile_pool(name="ring", bufs=RING_BUFS))

        xt = {}
        for s in ("L", "R"):
            xt[s] = big.tile([128, W0], f16, name="xt" + s)
        for s in ("L", "R"):
            nc.sync.dma_start(xt[s][0:20, 0:1024], xh_dram[s][0:20, 0:1024])
        wt = consts.tile([128, 512], f16)
        nc.scalar.dma_start(wt[:], wts_dram[:])
        bt = consts.tile([128, 2], f32)
        nc.scalar.dma_start(bt[:], bias_dram[:])
        actwarm = consts.tile([1, 2], f32)
        nc.scalar.activation(actwarm[:, 0:1], bt[0:1, 0:1], sigmoid)
        nc.scalar.activation(actwarm[:, 1:2], bt[0:1, 0:1], relu)
        for b in range(4):
            for s in ("L", "R"):
                if W_b[b] == 0:
                    continue
                c0 = 1024 if b == 0 else 0
                nc.sync.dma_start(
                    xt[s][32 * b : 32 * b + 20, c0 : W_b[b]],
                    xh_dram[s][20 * b : 20 * b + 20, c0 : W_b[b]],
                )

        # per-group alpha-space tiles so tree deps resolve at tile granularity
        h16g = [
            big.tile([128, 2 * n * m], f16, name=f"h16g{i}")
            for i, (m, j0, n, a0) in enumerate(groups)
        ]
        A_tot = groups[-1][3] + groups[-1][0] * groups[-1][2]
        pad_w = 2 * (A2 - A_tot)
        h16pad = big.tile([128, max(pad_w, 2)], f16, name="h16pad")
        NCHUNK = SEG_PER_CORE // 512
        pooled4c = [
            big.tile([128, 2 * FOLD * 512], f16, name=f"pooled4c{c}")
            for c in range(NCHUNK)
        ]
        hiddenc = [
            big.tile([32, 512], f16, name=f"hiddenc{c}") for c in range(NCHUNK)
        ]
        outb = big.tile([1, SEG_PER_CORE], f16, name="outb")

        # alpha -> (group index, a0, m, n, j0) lookup helpers
        def group_pieces(a_lo, a_hi):
            """Split [a_lo, a_hi) by group boundaries -> (gi|None, lo, hi)."""
            out = []
            pos = a_lo
            for i, (m, j0, n, a0) in enumerate(groups):
                g_lo, g_hi = a0, a0 + n * m
                if g_hi <= pos or g_lo >= a_hi:
                    continue
                lo, hi = max(pos, g_lo), min(a_hi, g_hi)
                if lo > pos:
                    out.append((None, pos, lo))
                out.append((i, lo, hi))
                pos = hi
            if pos < a_hi:
                out.append((None, pos, a_hi))
            return out

        def _tree_copy_one(out_ap, in_ap, cols, dve_ok=True, force_dve=False):
            dve_cost = cols * 0.26 + 70
            gps_cost = cols * 1.39 + 130
            if force_dve == "alt":
                eng_ns["late_i"] = eng_ns.get("late_i", 0) + 1
                force_dve = (eng_ns["late_i"] % ALT_MOD) != 0
            if not force_dve and (
                not dve_ok
                or eng_ns["gps"] + gps_cost < eng_ns["dve"] + dve_cost
            ):
                eng_ns["gps"] += gps_cost
                nc.gpsimd.tensor_copy(out_ap, in_ap)
            else:
                eng_ns["dve"] += dve_cost
                nc.vector.tensor_copy(out_ap, in_ap)

        def tree_copy(out_ap, in_ap, cols, n, dve_ok=True, force_dve=False):
            if cols <= TREE_CHUNK:
                _tree_copy_one(out_ap, in_ap, cols, dve_ok, force_dve)
                return
            pieces = -(-cols // TREE_CHUNK)
            step = -(-n // pieces)
            for n0 in range(0, n, step):
                n1 = min(n, n0 + step)
                _tree_copy_one(
                    out_ap[:, :, n0:n1], in_ap[:, :, n0:n1],
                    cols * (n1 - n0) // n, dve_ok, force_dve,
                )

        def emit_tree(gi_, m, j0, n, a0):
            sub = h16g[gi_][:].rearrange("p (s n m) -> p s n m", s=2, m=m)
            g_end = a0 + n * m
            late = g_end >= LATE_THR * A2
            dve_ok = late
            force_dve = "alt" if late else False
            w = m
            while w > 8:
                a = (w // 2) if w >= 17 else (w - 8)
                tree_tt(
                    sub[:, :, :, 0:a], sub[:, :, :, 0:a],
                    sub[:, :, :, w - a : w], 2 * n * a, a >= 2, n=n,
                    dve_ok=dve_ok, force_dve=force_dve,
                )
                w -= a
            # endgame into the per-chunk pooled tiles:
            #   w == 8      -> fold halves: q[0:4] = in[0:4] + in[4:8]
            #   w in 5..7   -> a=w-4 summed cols + copy of the 8-w middle
            #   w == 4      -> plain copy
            j = j0
            while j < j0 + n:
                c = j // 512
                j2 = min(j0 + n, (c + 1) * 512)
                p4 = pooled4c[c][:].rearrange(
                    "p (s j q) -> p s j q", s=2, q=FOLD
                )
                pout = p4[:, :, j - 512 * c : j2 - 512 * c, :]
                nn = j2 - j
                sv = sub[:, :, j - j0 : j2 - j0]
                if w == 8:
                    tree_tt(pout, sv[:, :, :, 0:4], sv[:, :, :, 4:8],
                            2 * nn * 4, True, n=nn, dve_ok=dve_ok, force_dve=force_dve)
                elif w == 4:
                    tree_copy(pout, sv[:, :, :, 0:4], 2 * nn * 4, nn,
                              dve_ok=dve_ok, force_dve=force_dve)
                else:
                    a = w - 4
                    tree_tt(pout[:, :, :, 0:a], sv[:, :, :, 0:a],
                            sv[:, :, :, w - a : w], 2 * nn * a, a >= 2,
                            n=nn, dve_ok=dve_ok, force_dve=force_dve)
                    tree_copy(pout[:, :, :, a:4], sv[:, :, :, a : w - a],
                              2 * nn * (8 - w), nn, dve_ok=dve_ok,
                              force_dve=force_dve)
                j = j2

        pp = ctx.enter_context(tc.tile_pool(name="psum", bufs=8, space="PSUM"))
        if True:
            sti = 0
            for k in range(n_st):
                b = int(np.searchsorted(st_edges, k, side="right") - 1)
                colA = (k - st_edges[b]) * 1024
                colB = colA + 512
                wap = wt[32 * b : 32 * b + 20, 0:128]
                for s_i, s in enumerate(("L", "R")):
                    stt = STT_PATTERN[sti % len(STT_PATTERN)]
                    sti += 1
                    ptB = pp.tile([128, 512], f32, tag="pt")
                    nc.tensor.matmul(
                        ptB[:], wap,
                        xt[s][32 * b : 32 * b + 20, colB : colB + 512],
                        start=True, stop=True, tile_position=(32 * b, 0),
                    )
                    ptA = pp.tile([128, 512], f32, tag="pt")
                    nc.tensor.matmul(
                        ptA[:], wap,
                        xt[s][32 * b : 32 * b + 20, colA : colA + 512],
                        start=True, stop=True, tile_position=(32 * b, 0),
                    )
                    rg = ring_pool.tile([128, 512], f16, tag="ring")
                    nc.scalar.activation(rg[:], ptB[:], relu)
                    for (g_i, lo, hi) in group_pieces(512 * k, 512 * k + 512):
                        if g_i is None:
                            dst = h16pad[
                                :, s_i * (A2 - A_tot) + lo - A_tot
                                : s_i * (A2 - A_tot) + hi - A_tot
                            ]
                        else:
                            a0g = groups[g_i][3]
                            dst = h16g[g_i][
                                :, s_i * groups[g_i][0] * groups[g_i][2]
                                + lo - a0g
                                : s_i * groups[g_i][0] * groups[g_i][2]
                                + hi - a0g
                            ]
                        pa = ptA[:, lo - 512 * k : hi - 512 * k]
                        rga = rg[:, lo - 512 * k : hi - 512 * k]
                        if stt:
                            nc.vector.scalar_tensor_tensor(
                                dst, pa, 0.0, rga, mx, add
                            )
                        else:
                            nc.scalar.activation(dst, pa, relu)
                            nc.vector.tensor_tensor(dst, dst, rga, add)
            for g_i, g in enumerate(groups):
                emit_tree(g_i, *g)

            # combine MLP: 4 pooled cols/seg stream as FOLD accumulating
            # passes; psum tiles come from the same pool (same tag) so no
            # pool-close drain separates combine from the transit.
            ph_ts = []
            for c in range(NCHUNK):
                p4v = pooled4c[c][:].rearrange(
                    "p (s j q) -> p s j q", s=2, q=FOLD
                )
                ph_t = pp.tile([128, 512], f32, tag="pt")
                ph_ts.append(ph_t)
                nmm = 2 * FOLD
                i = 0
                for s_i in range(2):
                    wc = wt[:, 128 + 128 * s_i : 256 + 128 * s_i]
                    for qq in range(FOLD):
                        nc.tensor.matmul(
                            ph_t[:], wc, p4v[:, s_i, :, qq],
                            start=(i == 0), stop=(i == nmm - 1),
                        )
                        i += 1
                nc.scalar.activation(
                    hiddenc[c][:], ph_t[0:32, :], relu, bias=bt[0:32, 0:1]
                )
            pl_ts = []
            for c in range(NCHUNK):
                pl_t = pp.tile([128, 512], f32, tag="pt")
                pl_ts.append(pl_t)
                nc.tensor.matmul(
                    pl_t[:], wt[0:32, 384:512], hiddenc[c][:],
                    start=True, stop=True,
                )
            for c in range(NCHUNK):
                nc.scalar.activation(
                    outb[:, 512 * c : 512 * c + 512], pl_ts[c][0:1, :],
                    sigmoid, bias=bt[0:1, 1:2],
                )
                nc.sync.dma_start(
                    out_dram[:, 512 * c : 512 * c + 512],
                    outb[:, 512 * c : 512 * c + 512],
                )

    nc.compile()
    return nc


# ------------------------------------------------------------------- driver

def kernel(**inputs):
    meta = host_prep(
        inputs["left_feats"], inputs["right_feats"],
        inputs["left_seg"], inputs["right_seg"],
    )
    wab = make_weight_arrays(
        inputs["W1"], inputs["b1"], inputs["Wc1"], inputs["bc1"],
        inputs["Wc2"], inputs["bc2"],
    )
    nc = build_nc(meta["A2"], meta["n_st"], meta["aux"], meta["groups"])
    in_maps = []
    for d in range(N_CORES):
        c = meta["cores"][d]
        in_maps.append(dict(xhL=c["xhL"], xhR=c["xhR"], wts=wab["wts"],
                            bias=wab["bias"]))
    res = run_bass_kernel_spmd(nc, in_maps, core_ids=list(range(N_CORES)))
    global _last_results
    _last_results = res
    out = np.zeros(BATCH, dtype=np.float32)
    for d in range(N_CORES):
        order = meta["cores"][d]["order"]
        dev = np.asarray(res.results[d]["out"]).reshape(-1).astype(np.float32)
        out[SEG_PER_CORE * d + order] = dev
    return out

